# revision 24
# baseline (speedup 1.0000x reference)
"""Trainium2 Bass kernel for nn_A2CDense (GNN message passing A2C net).

8 NeuronCores = 4 batch-pairs x 2 roles (even core: value head, odd: policy).
Both cores of a pair stream one half of Es[b]/Er[b] (512 MiB total read once =
memory roofline), convert the one-hot incidence columns to int16 indices with
iota-matmuls on the TensorEngine, exchange index halves via a 2-rank
AllGather, then run the GN blocks with dma_gather (bf16 SBUF gathers) and
dma_scatter_add (f32 CCE scatter-add to HBM) instead of dense incidence
matmuls.  Both roles run the same SPMD graph; role differences live in the
weight blobs (policy block2 edge-MLP zero-padded; value head expressed as a
relu-pair linear passthrough).
"""

import sys

for _p in ("/opt/trn_rl_repo", "/root/.axon_site"):
    if _p not in sys.path:
        sys.path.insert(0, _p)

import numpy as np
import ml_dtypes

import concourse.bass as bass
import concourse.mybir as mybir
from concourse import bacc
from concourse.tile import TileContext
from concourse.bass_utils import run_bass_kernel_spmd

FP32 = mybir.dt.float32
BF16 = mybir.dt.bfloat16
I16 = mybir.dt.int16
I32 = mybir.dt.int32
U8 = mybir.dt.uint8
AF = mybir.ActivationFunctionType
ALU = mybir.AluOpType
AX = mybir.AxisListType

B_ = 4
N_ = 2048
E_ = 8192
H = 64
IV = IE = IU = 32
VA_IN = 20
OUT_EA = 194
MAXGRID = 50.0


class Cfg:
    def __init__(self, N=N_, E=E_, n_cores=8, segs=None):
        # E here is the DEVICE edge count (with pad slots); must be 256-mult
        assert N % 128 == 0 and E % 256 == 0
        self.N, self.E = N, E
        self.EH = E // 2
        self.NCH = N // 128
        self.n_cores = n_cores
        # segs: list of (bank_q, capacity) in slot order, grouped in rounds of 4
        self.segs = segs or []
        # rounds: list of sizes (sum of its segs' caps)
        self.rounds = []
        i = 0
        while i < len(self.segs):
            r = self.segs[i:i + 4]
            self.rounds.append(sum(c for _, c in r))
            i += 4

    def blk(self, k):
        if k == 0:
            return dict(fe_in=4, fv=VA_IN, fu_in=0)
        return dict(fe_in=IE, fv=IV, fu_in=IU)


# ------------------------------------------------------------- weight planning
class Plan:
    def __init__(self):
        self.cols = {"f32": 0, "bf16": 0}
        self.slots = {}

    def add(self, name, rows, cols, kind="f32"):
        off = self.cols[kind]
        self.slots[name] = (kind, off, rows, cols)
        self.cols[kind] = off + cols

    def sl(self, sb_f32, sb_bf16, name):
        kind, off, rows, cols = self.slots[name]
        t = sb_f32 if kind == "f32" else sb_bf16
        return t[0:rows, off:off + cols]


def make_plan(cfg):
    p = Plan()
    for k in range(3):
        d = cfg.blk(k)
        fe_in, fv, fu_in = d["fe_in"], d["fv"], d["fu_in"]
        p.add(f"W1e{k}", fe_in, H, "bf16")
        p.add(f"W1s{k}", fv, H, "bf16")
        p.add(f"W1r{k}", fv, H, "bf16")
        if fu_in:
            p.add(f"W1u{k}", fu_in, H)
            p.add(f"W1uv{k}", fu_in, H)
        p.add(f"b1e{k}", H, 1)
        p.add(f"W2e{k}", H, H); p.add(f"b2e{k}", H, 1)
        p.add(f"W3e{k}", H, H); p.add(f"b3e{k}", H, 1)
        p.add(f"W4e{k}", H, IE); p.add(f"b4e{k}", 1, IE)
        p.add(f"W1v{k}", fv, H)
        p.add(f"W1g{k}", IE, H)
        p.add(f"b1v{k}", H, 1)
        p.add(f"W2v{k}", H, H); p.add(f"b2v{k}", H, 1)
        p.add(f"W3v{k}", H, H); p.add(f"b3v{k}", H, 1)
        p.add(f"W4v{k}", H, IV); p.add(f"b4v{k}", 1, IV)
        p.add(f"W1u_{k}", 96, H); p.add(f"b1u{k}", H, 1)
        p.add(f"W2u{k}", H, H); p.add(f"b2u{k}", H, 1)
        p.add(f"W3u{k}", H, H); p.add(f"b3u{k}", H, 1)
        p.add(f"W4u{k}", H, IU); p.add(f"b4u{k}", 1, IU)
    p.add("Wh1", 96, H); p.add("bh1", H, 1)
    p.add("Wh2", H, H); p.add("bh2", H, 1)
    p.add("Wh3", H, H); p.add("bh3", H, 1)
    p.add("Wh4", H, OUT_EA)
    p.add("bh4", 1, OUT_EA)
    return p


class Consts:
    IOTA = 0            # [128, 16] global node iota chunks (f32)
    IDENT = IOTA + 16   # [128, 128] identity
    ONES = IDENT + 128  # [128, 1] ones column
    ONESROW = ONES + 1  # ones row on partition 0, 512 wide
    I9 = ONESROW + 512
    I3 = I9 + 1
    I6 = I3 + 1
    I4 = I6 + 1
    TOTAL = I4 + 1


def host_consts(cfg):
    c = np.zeros((128, Consts.TOTAL), np.float32)
    for k in range(cfg.NCH):
        c[:, Consts.IOTA + k] = np.arange(128 * k, 128 * (k + 1))
    c[:, Consts.IDENT:Consts.IDENT + 128] = np.eye(128)
    c[:, Consts.ONES] = 1.0
    c[0, Consts.ONESROW:Consts.ONESROW + 512] = 1.0
    c[:, Consts.I9:Consts.I4 + 1] = -100.0
    c[:9, Consts.I9] = np.arange(9) - 1
    c[:3, Consts.I3] = np.arange(3) - 1
    c[:6, Consts.I6] = np.arange(6)
    c[:4, Consts.I4] = np.arange(4)
    return c


def _np(x):
    return np.asarray(x, np.float32)


def plan_edges(Er):
    """Compute rank-round/bank-quarter segment capacities and per-batch slot
    assignment from the receiver structure.  Returns (segs, E_dev, perms)
    where perms[b][slot] = original edge id or -1 (pad)."""
    B = Er.shape[0]
    E = Er.shape[2]
    rcv = np.asarray(Er).argmax(1)          # [B, E]
    ranks = np.zeros((B, E), np.int64)
    for b in range(B):
        order = np.argsort(rcv[b], kind="stable")
        sr = rcv[b][order]
        first = np.ones(E, bool)
        first[1:] = sr[1:] != sr[:-1]
        idx_first = np.maximum.accumulate(np.where(first, np.arange(E), 0))
        ranks[b, order] = np.arange(E) - idx_first
    maxrank = int(ranks.max())
    R = maxrank // 4 + 1
    caps = []
    for r in range(R):
        for q in range(4):
            cnt = max(int(((ranks // 4 == r) & (ranks % 4 == q))[b].sum()) for b in range(B))
            caps.append((q, max(128, -(-cnt // 128) * 128) if cnt > 0 else 0))
    # drop trailing zero segments but keep round grouping (pad zero caps inside)
    segs = [(q, c) for (q, c) in caps]
    while segs and segs[-1][1] == 0:
        segs.pop()
    # pad segs to multiple of 4 entries with zero caps for clean rounds
    while len(segs) % 4 != 0:
        segs.append((len(segs) % 4, 0))
    E_dev = sum(c for _, c in segs)
    if E_dev % 256 != 0:
        q, c = segs[-1]
        segs[-1] = (q, c + 256 - E_dev % 256)
        E_dev = sum(c for _, c in segs)
    perms = np.full((B, E_dev), -1, np.int64)
    for b in range(B):
        off = 0
        for si, (q, c) in enumerate(segs):
            r = si // 4
            sel = np.nonzero((ranks[b] // 4 == r) & (ranks[b] % 4 == q))[0]
            assert len(sel) <= c, (si, len(sel), c)
            perms[b, off:off + len(sel)] = sel
            off += c
    return segs, E_dev, perms


def host_weights(cfg, params, role):
    plan = make_plan(cfg)
    wf = np.zeros((128, plan.cols["f32"]), np.float32)
    wb = np.zeros((128, plan.cols["bf16"]), np.float32)

    def put(name, arr):
        kind, off, rows, cols = plan.slots[name]
        t = wf if kind == "f32" else wb
        t[:rows, off:off + cols] = _np(arr).reshape(rows, cols)

    trunk = params["val"] if role == 0 else params["pol"]
    for k in range(3):
        d = cfg.blk(k)
        fe_in, fv, fu_in = d["fe_in"], d["fv"], d["fu_in"]
        blkp = trunk[k]
        zero_fe = (role == 1 and k == 2)
        if zero_fe:
            put(f"W1e{k}", np.zeros((fe_in, H)))
            put(f"W1s{k}", np.zeros((fv, H)))
            put(f"W1r{k}", np.zeros((fv, H)))
            put(f"W1u{k}", np.zeros((fu_in, H)))
            put(f"b1e{k}", np.zeros((H, 1)))
            put(f"W2e{k}", np.zeros((H, H))); put(f"b2e{k}", np.zeros((H, 1)))
            put(f"W3e{k}", np.zeros((H, H))); put(f"b3e{k}", np.zeros((H, 1)))
            put(f"W4e{k}", np.zeros((H, IE))); put(f"b4e{k}", np.zeros((1, IE)))
        else:
            W1, b1 = blkp["fe"][0]
            W1 = _np(W1)
            put(f"W1e{k}", W1[:fe_in])
            put(f"W1s{k}", W1[fe_in:fe_in + fv])
            put(f"W1r{k}", W1[fe_in + fv:fe_in + 2 * fv])
            if fu_in:
                put(f"W1u{k}", W1[fe_in + 2 * fv:])
            put(f"b1e{k}", _np(b1).reshape(H, 1))
            for li, nm in ((1, "2e"), (2, "3e")):
                Wx, bx = blkp["fe"][li]
                put(f"W{nm}{k}", Wx); put(f"b{nm}{k}", _np(bx).reshape(H, 1))
            W4, b4 = blkp["fe"][3]
            put(f"W4e{k}", W4); put(f"b4e{k}", _np(b4).reshape(1, IE))
        # fv rows: [V(fv), agg(IE), u(fu_in)]
        W1, b1 = blkp["fv"][0]
        W1 = _np(W1)
        put(f"W1v{k}", W1[:fv])
        if role == 1 and k == 2:
            put(f"W1g{k}", np.zeros((IE, H)))
            put(f"W1uv{k}", W1[fv:])
        else:
            put(f"W1g{k}", W1[fv:fv + IE])
            if fu_in:
                put(f"W1uv{k}", W1[fv + IE:])
        put(f"b1v{k}", _np(b1).reshape(H, 1))
        for li, nm in ((1, "2v"), (2, "3v")):
            Wx, bx = blkp["fv"][li]
            put(f"W{nm}{k}", Wx); put(f"b{nm}{k}", _np(bx).reshape(H, 1))
        W4, b4 = blkp["fv"][3]
        put(f"W4v{k}", W4); put(f"b4v{k}", _np(b4).reshape(1, IV))
        # fu rows padded to [u(32), sum_V(32), sum_E(32)]
        W1, b1 = blkp["fu"][0]
        W1 = _np(W1)
        W1p = np.zeros((96, H), np.float32)
        if role == 1 and k == 2:
            W1p[0:32] = W1[:32]
            W1p[32:64] = W1[32:64]
        elif k == 0:
            W1p[32:64] = W1[:32]
            W1p[64:96] = W1[32:64]
        else:
            W1p[:] = W1
        put(f"W1u_{k}", W1p); put(f"b1u{k}", _np(b1).reshape(H, 1))
        for li, nm in ((1, "2u"), (2, "3u")):
            Wx, bx = blkp["fu"][li]
            put(f"W{nm}{k}", Wx); put(f"b{nm}{k}", _np(bx).reshape(H, 1))
        W4, b4 = blkp["fu"][3]
        put(f"W4u{k}", W4); put(f"b4u{k}", _np(b4).reshape(1, IU))

    if role == 1:
        po = params["out"]["fe"]
        for li, nm in ((0, "h1"), (1, "h2"), (2, "h3")):
            Wx, bx = po[li]
            put(f"W{nm}", Wx); put(f"b{nm}", _np(bx).reshape(H, 1))
        W4, b4 = po[3]
        put("Wh4", W4); put("bh4", _np(b4).reshape(1, OUT_EA))
    else:
        Wv, bv = params["lin_val"]
        Wv = _np(Wv).reshape(IU)
        W1 = np.zeros((96, H), np.float32)
        W1[64:96, 0:32] = np.eye(32)
        W1[64:96, 32:64] = -np.eye(32)
        put("Wh1", W1); put("bh1", np.zeros((H, 1)))
        put("Wh2", np.eye(H)); put("bh2", np.zeros((H, 1)))
        put("Wh3", np.eye(H)); put("bh3", np.zeros((H, 1)))
        W4 = np.zeros((H, OUT_EA), np.float32)
        W4[0:32, 0] = Wv
        W4[32:64, 0] = -Wv
        b4 = np.zeros((1, OUT_EA), np.float32)
        b4[0, 0] = float(np.asarray(bv).reshape(-1)[0])
        put("Wh4", W4); put("bh4", b4)

    return wf, wb.astype(ml_dtypes.bfloat16), plan


# ------------------------------------------------------------------ builder
def build_nc(cfg):
    N, E, EH = cfg.N, cfg.E, cfg.EH
    NCH = cfg.NCH
    plan = make_plan(cfg)

    nc = bacc.Bacc(None, target_bir_lowering=False)

    EsH = nc.declare_dram_parameter("EsH", [N, EH], FP32, isOutput=False)
    ErH = nc.declare_dram_parameter("ErH", [N, EH], FP32, isOutput=False)
    VaD = nc.declare_dram_parameter("Va", [5, N], I32, isOutput=False)
    EaD = nc.declare_dram_parameter("Ea", [1, E], I32, isOutput=False)
    mskE = nc.declare_dram_parameter("mask_e", [1, E], U8, isOutput=False)
    mskH = nc.declare_dram_parameter("mask_h", [1, 256], U8, isOutput=False)
    offsD = nc.declare_dram_parameter("offs_w", [128, E // 16], I16, isOutput=False)
    wf32D = nc.declare_dram_parameter("wf32", [128, plan.cols["f32"]], FP32, isOutput=False)
    wb16D = nc.declare_dram_parameter("wbf16", [128, plan.cols["bf16"]], BF16, isOutput=False)
    constD = nc.declare_dram_parameter("consts", [128, Consts.TOTAL], FP32, isOutput=False)
    outD = nc.declare_dram_parameter("out", [2, 256], FP32, isOutput=True)

    with TileContext(nc) as tc:
        with (
            tc.tile_pool(name="dram", bufs=1, space="DRAM") as dramp,
            tc.tile_pool(name="res", bufs=1) as res,
            tc.tile_pool(name="stream", bufs=4) as streamp,
            tc.tile_pool(name="tmp", bufs=2) as tmpp,
            tc.tile_pool(name="gath", bufs=1) as gathp,
            tc.tile_pool(name="hbuf", bufs=6) as hbufp,
            tc.tile_pool(name="oute", bufs=2) as outep,
            tc.tile_pool(name="vt", bufs=1) as vtp,
            tc.tile_pool(name="small", bufs=2) as smallp,
            tc.tile_pool(name="ps_conv", bufs=2, space="PSUM") as ps_conv,
            tc.tile_pool(name="ps_a", bufs=2, space="PSUM") as ps_a,
            tc.tile_pool(name="ps_b", bufs=2, space="PSUM") as ps_b,
            tc.tile_pool(name="ps_t", bufs=2, space="PSUM") as ps_t,
        ):
            # ------------- resident
            wf = res.tile([128, plan.cols["f32"]], FP32)
            wb = res.tile([128, plan.cols["bf16"]], BF16)
            cst = res.tile([128, Consts.TOTAL], FP32)
            nc.sync.dma_start(wf[:], wf32D[:])
            nc.sync.dma_start(wb[:], wb16D[:])
            nc.sync.dma_start(cst[:], constD[:])

            def W(name):
                return plan.sl(wf, wb, name)

            ident = cst[:, Consts.IDENT:Consts.IDENT + 128]
            ones_col = cst[:, Consts.ONES:Consts.ONES + 1]
            ones_row = cst[0:1, Consts.ONESROW:Consts.ONESROW + 512]
            identb = res.tile([128, 128], BF16)
            nc.scalar.copy(identb[:], ident)

            # ------------- setup: encodes + masks
            va_enc = res.tile([32, N], FP32)
            nc.vector.memset(va_enc[:], 0.0)
            nqc = min(512, N)
            for q0 in range(0, N, nqc):
                for rows, src_row, icol, dst0 in ((9, 0, Consts.I9, 0), (3, 1, Consts.I3, 9), (6, 4, Consts.I6, 14)):
                    rep = tmpp.tile([16, nqc], I32, tag="tmp", name="rep")
                    nc.sync.dma_start(rep[0:rows], VaD[src_row:src_row + 1, q0:q0 + nqc].broadcast_to([rows, nqc]))
                    enc = tmpp.tile([16, nqc], FP32, tag="tmpf", name="enc")
                    nc.vector.tensor_scalar(enc[0:rows], rep[0:rows], cst[0:rows, icol:icol + 1], None, ALU.is_equal)
                    nc.sync.dma_start(va_enc[dst0:dst0 + rows, q0:q0 + nqc], enc[0:rows])
                cxy_i = tmpp.tile([16, nqc], I32, tag="tmp", name="cxy_i")
                nc.sync.dma_start(cxy_i[0:2], VaD[2:4, q0:q0 + nqc])
                cxy_f = tmpp.tile([16, nqc], FP32, tag="tmpf", name="cxy_f")
                nc.vector.tensor_scalar(cxy_f[0:2], cxy_i[0:2], 2.0 / MAXGRID, -1.0, ALU.mult, ALU.add)
                nc.sync.dma_start(va_enc[12:14, q0:q0 + nqc], cxy_f[0:2])

            ea_enc = res.tile([4, E], BF16)
            me32 = res.tile([32, E], BF16)
            eqc = min(512, E)
            for q0 in range(0, E, eqc):
                w = min(eqc, E - q0)
                ear = tmpp.tile([16, eqc], I32, tag="tmp", name="ear")
                nc.sync.dma_start(ear[0:4, 0:w], EaD[0:1, q0:q0 + w].broadcast_to([4, w]))
                nc.vector.tensor_scalar(ea_enc[:, q0:q0 + w], ear[0:4, 0:w], cst[0:4, Consts.I4:Consts.I4 + 1], None, ALU.is_equal)
                mer = tmpp.tile([32, eqc], U8, tag="tmpu", name="mer")
                nc.sync.dma_start(mer[:, 0:w], mskE[0:1, q0:q0 + w].broadcast_to([32, w]))
                nc.vector.tensor_copy(me32[:, q0:q0 + w], mer[:, 0:w])

            mh_u8 = res.tile([1, 256], U8)
            nc.sync.dma_start(mh_u8[:], mskH[:])
            mh_f = res.tile([1, 256], FP32)
            nc.vector.tensor_copy(mh_f[:], mh_u8[:])

            zt = res.tile([128, 1024], FP32)
            nc.vector.memset(zt[:], 0.0)
            NAGG = 8 * N
            agg_d = [dramp.tile([NAGG, 64], FP32, tag=f"agg{k}", name=f"agg{k}") for k in range(3)]
            zcover = min(1024 // 64 * 128, NAGG)
            for k in range(3):
                for z0 in range(0, NAGG, zcover):
                    nc.sync.dma_start(
                        agg_d[k][z0:z0 + zcover].rearrange("(s p) f -> p s f", p=128),
                        zt[:, 0:(zcover // 128) * 64].rearrange("p (s f) -> p s f", f=64))

            # ------------- phase 1: stream + convert
            cc_in = dramp.tile([2, EH], I16, tag="ccin")
            cc_out = dramp.tile([4, EH], I16, tag="ccout")
            for mi, src in enumerate((EsH, ErH)):
                for j0 in range(0, EH, 512):
                    w = min(512, EH - j0)
                    pchain = ps_conv.tile([1, 512], FP32, tag="conv", name="pchain")
                    for i in range(NCH):
                        st = streamp.tile([128, 512], FP32, tag="stream", name="st")
                        nc.sync.dma_start(st[:, 0:w], src[128 * i:128 * (i + 1), j0:j0 + w])
                        nc.tensor.matmul(pchain[:, 0:w], cst[:, Consts.IOTA + i:Consts.IOTA + i + 1], st[:, 0:w],
                                         start=(i == 0), stop=(i == NCH - 1))
                    stg = smallp.tile([1, 512], I16, tag="idxstg", name="stg")
                    nc.vector.tensor_copy(stg[:, 0:w], pchain[:, 0:w])
                    nc.sync.dma_start(cc_in[mi:mi + 1, j0:j0 + w], stg[:, 0:w])
            rg = [[2 * i, 2 * i + 1] for i in range(cfg.n_cores // 2)]
            nc.gpsimd.collective_compute(
                "AllGather", ALU.bypass, ins=[cc_in[:].opt()], outs=[cc_out[:].opt()],
                replica_groups=rg)

            snd_w = res.tile([128, E // 16], I16)
            rcv_w = res.tile([128, E // 16], I16)
            for arr, row in ((snd_w, 0), (rcv_w, 1)):
                for hh in range(2):
                    nc.sync.dma_start(
                        arr[0:16, (EH // 16) * hh:(EH // 16) * (hh + 1)],
                        cc_out[row + 2 * hh:row + 2 * hh + 1, :].rearrange("o (s p) -> (o p) s", p=16))
                for p in (16, 32, 64):
                    nc.sync.dma_start(arr[p:2 * p], arr[0:p])

            # banked scatter idx: rcv_sc = rcv_w + seg_off + 4N*(1-mask_e)
            offs_sb = res.tile([128, E // 16], I16)
            nc.sync.dma_start(offs_sb[:], offsD[:])
            mw_u8 = res.tile([16, E // 16], U8, tag="mwu8")
            nc.sync.dma_start(mw_u8[:], mskE[0:1, :].rearrange("o (s p) -> (o p) s", p=16))
            mw_f = res.tile([16, E // 16], FP32, tag="mwf")
            nc.vector.tensor_scalar(mw_f[:], mw_u8[:], float(-4 * N), float(4 * N), ALU.mult, ALU.add)
            mw_i = res.tile([128, E // 16], I16, tag="mwi")
            nc.vector.tensor_copy(mw_i[0:16], mw_f[:])
            for p in (16, 32, 64):
                nc.sync.dma_start(mw_i[p:2 * p], mw_i[0:p])
            rcv_sc = res.tile([128, E // 16], I16)
            nc.vector.tensor_tensor(rcv_sc[:], rcv_w[:], offs_sb[:], ALU.add)
            nc.vector.tensor_tensor(rcv_sc[:], rcv_sc[:], mw_i[:], ALU.add)

            # round boundaries (slot positions)
            rb_pos = []
            acc = 0
            for rsz in cfg.rounds:
                if rsz:
                    rb_pos.append((acc, acc + rsz))
                    acc += rsz

            # ------------- block machinery
            def vt_prep(V, fv):
                vt = vtp.tile([128, NCH * 128], BF16, tag="vt")
                nc.vector.memset(vt[:], 0.0)
                for g in range(0, NCH, 4):
                    ng = min(4, NCH - g)
                    pt = ps_t.tile([128, 128], FP32, tag="t")
                    for t in range(ng):
                        nc.tensor.transpose(pt[:, 32 * t:32 * t + fv],
                                            V[0:fv, 128 * (g + t):128 * (g + t + 1)],
                                            ident[0:fv, 0:fv])
                    src_v = pt[:, 0:32 * ng].rearrange("p (s c) -> p s c", c=32)[:, :, 0:fv]
                    dst_v = vt[:, 128 * g:128 * (g + ng)].rearrange("p (s c) -> p s c", c=128)[:, :, 0:fv]
                    nc.scalar.copy(dst_v, src_v)
                return vt

            def gather(vt, idx_w, label, ne):
                g = gathp.tile([128, ne], BF16, tag=f"g_{label}", name=f"g{label}")
                nc.gpsimd.dma_gather(
                    g[:].rearrange("p (c e) -> p c e", c=1), vt[:],
                    idx_w, ne, ne, 128,
                    transpose=True, sbuf_tokens_per_rank=128,
                    sbuf_free_dim_per_rank=256, single_packet=False)
                return g

            def mlp_ev(pin, bias_ap, out_ap, engine):
                if engine == "act":
                    nc.scalar.activation(out_ap, pin, AF.Relu, bias=bias_ap, scale=1.0)
                else:
                    nc.vector.tensor_scalar(out_ap, pin, bias_ap, 0.0, ALU.add, ALU.max)

            def col_mlp(x96, names, out_dim):
                h = None
                for li in range(3):
                    wn, bn = names[li]
                    pt = ps_b.tile([H, 1], FP32, tag="b")
                    nc.tensor.matmul(pt[:], W(wn), x96 if li == 0 else h[:], start=True, stop=True)
                    hn = smallp.tile([H, 1], FP32, tag=f"colh{li}")
                    nc.scalar.activation(hn[:], pt[:], AF.Relu, bias=W(bn), scale=1.0)
                    h = hn
                wn, bn = names[3]
                pr = ps_b.tile([1, out_dim], FP32, tag="b")
                nc.tensor.matmul(pr[:], h[:], W(wn), start=True, stop=False)
                nc.tensor.matmul(pr[:], cst[0:1, Consts.ONES:Consts.ONES + 1], W(bn), start=False, stop=True)
                return pr

            V_cur = va_enc
            Ea_cur = ea_enc
            u_cur = None

            for k in range(3):
                d = cfg.blk(k)
                fe_in, fv, fu_in = d["fe_in"], d["fv"], d["fu_in"]

                if fu_in:
                    bfold = ps_b.tile([H, 2], FP32, tag="b")
                    nc.tensor.matmul(bfold[:, 0:1], W(f"W1u{k}"), u_cur[:], start=True, stop=True)
                    nc.tensor.matmul(bfold[:, 1:2], W(f"W1uv{k}"), u_cur[:], start=True, stop=True)
                    b1eff = smallp.tile([H, 2], FP32, tag="b1eff")
                    nc.vector.tensor_tensor(b1eff[:, 0:1], bfold[:, 0:1], W(f"b1e{k}"), ALU.add)
                    nc.vector.tensor_tensor(b1eff[:, 1:2], bfold[:, 1:2], W(f"b1v{k}"), ALU.add)
                    b1e_ap, b1v_ap = b1eff[:, 0:1], b1eff[:, 1:2]
                else:
                    b1e_ap, b1v_ap = W(f"b1e{k}"), W(f"b1v{k}")

                vt = vt_prep(V_cur, fv)

                # ---- edge MLP + scatter, in two E/2 passes ----
                oute = outep.tile([32, E], BF16, tag="oute")
                n_half = E // 2
                hch = n_half // 512
                wcols = n_half // 16      # wrapped idx cols per half
                sumE_parts = smallp.tile([1, 8 * 32], FP32, tag="sumep", bufs=1)
                nsum = 0
                for hf in range(2):
                    e0 = hf * n_half
                    gs = gather(vt, snd_w[:, hf * wcols:(hf + 1) * wcols], "s", n_half)
                    gr = gather(vt, rcv_w[:, hf * wcols:(hf + 1) * wcols], "r", n_half)
                    for c0 in range(0, n_half, 512):
                        cw = min(512, n_half - c0)
                        sl = slice(e0 + c0, e0 + c0 + cw)
                        lsl = slice(c0, c0 + cw)
                        p1 = ps_a.tile([H, 512], FP32, tag="a", name="p1e")
                        nc.tensor.matmul(p1[:, 0:cw], W(f"W1e{k}"), Ea_cur[0:fe_in, sl], start=True, stop=False)
                        nc.tensor.matmul(p1[:, 0:cw], W(f"W1s{k}"), gs[0:fv, lsl], start=False, stop=False)
                        nc.tensor.matmul(p1[:, 0:cw], W(f"W1r{k}"), gr[0:fv, lsl], start=False, stop=True)
                        h1 = hbufp.tile([H, 512], FP32, tag="h", name="h1")
                        mlp_ev(p1[:, 0:cw], b1e_ap, h1[:, 0:cw], "act")
                        p2 = ps_a.tile([H, 512], FP32, tag="a", name="p2e")
                        nc.tensor.matmul(p2[:, 0:cw], W(f"W2e{k}"), h1[:, 0:cw], start=True, stop=True)
                        h2 = hbufp.tile([H, 512], FP32, tag="h", name="h2")
                        mlp_ev(p2[:, 0:cw], W(f"b2e{k}"), h2[:, 0:cw], "dve")
                        p3 = ps_a.tile([H, 512], FP32, tag="a", name="p3e")
                        nc.tensor.matmul(p3[:, 0:cw], W(f"W3e{k}"), h2[:, 0:cw], start=True, stop=True)
                        h3 = hbufp.tile([H, 512], FP32, tag="h", name="h3")
                        mlp_ev(p3[:, 0:cw], W(f"b3e{k}"), h3[:, 0:cw], "act")
                        p4 = ps_b.tile([32, 512], FP32, tag="b", name="p4e")
                        nc.tensor.matmul(p4[:, 0:cw], W(f"W4e{k}"), h3[:, 0:cw], start=True, stop=False)
                        nc.tensor.matmul(p4[:, 0:cw], W(f"b4e{k}"), ones_row[:, 0:cw], start=False, stop=True)
                        nc.vector.tensor_tensor(oute[:, sl], p4[:, 0:cw], me32[:, sl], ALU.mult)

                    # transpose masked out_E half -> [Ewrap, 32] f32
                    ssrc = gathp.tile([128, (n_half // 128) * 32], FP32, tag="ssrc", name="ssrc")
                    for c0 in range(0, n_half, 512):
                        cw = min(512, n_half - c0)
                        pt = ps_t.tile([128, 128], BF16, tag="t", name="ptb")
                        for t in range(0, cw, 128):
                            ee = e0 + c0 + t
                            nc.tensor.transpose(pt[:, (t // 128) * 32:(t // 128) * 32 + 32],
                                                oute[:, ee:ee + 128], identb[0:32, 0:32])
                        nc.vector.tensor_copy(ssrc[:, (c0 // 128) * 32:((c0 + cw) // 128) * 32],
                                              pt[:, 0:(cw // 128) * 32])
                    # scatter per (round x half) intersection: unique receivers per call
                    for (r0_, r1_) in rb_pos:
                        a0, a1 = max(r0_, e0), min(r1_, e0 + n_half)
                        if a0 >= a1:
                            continue
                        nidx = a1 - a0
                        nc.gpsimd.dma_scatter_add(
                            agg_d[k][:, 0:32],
                            ssrc[:, ((a0 - e0) // 128) * 32:((a1 - e0) // 128) * 32].rearrange(
                                "p (s f) -> p s f", f=32),
                            rcv_sc[:, a0 // 16:a1 // 16], nidx, nidx, 32, elem_step=64,
                            single_packet=False)
                    ssrc_cols = (n_half // 128) * 32
                    for q in range(0, ssrc_cols, 512):
                        qc = min(512, ssrc_cols - q)
                        pq = ps_conv.tile([1, 512], FP32, tag="conv", name="pq")
                        nc.tensor.matmul(pq[:, 0:qc], ones_col, ssrc[:, q:q + qc], start=True, stop=True)
                        nc.vector.tensor_reduce(
                            sumE_parts[0:1, 32 * nsum:32 * (nsum + 1)],
                            pq[:, 0:qc].rearrange("o (s f) -> o f s", f=32), AX.X, ALU.add)
                        nsum += 1
                sumE_row = smallp.tile([1, 32], FP32, tag="sume")
                nc.vector.tensor_reduce(
                    sumE_row[:], sumE_parts[0:1, 0:32 * nsum].rearrange("o (q f) -> o f q", f=32),
                    AX.X, ALU.add)
                psE = ps_t.tile([32, 1], FP32, tag="t", name="psE")
                nc.tensor.transpose(psE[:], sumE_row[:], ident[0:1, 0:1])
                sumE_col = smallp.tile([32, 1], FP32, tag="sumec")
                nc.vector.tensor_copy(sumE_col[:], psE[:])

                # ---- agg readback (4 banks) + sum + transpose ----
                agg_rb = gathp.tile([128, NCH * 32], FP32, tag="aggrb")
                rb_a = gathp.tile([128, NCH * 32], FP32, tag="aggrbx", bufs=2, name="rb_a")
                nc.sync.dma_start(
                    rb_a[:].rearrange("p (s f) -> p s f", f=32),
                    agg_d[k][0:N, 0:32].rearrange("(s p) f -> p s f", p=128))
                for j in range(1, 4):
                    rb_b = gathp.tile([128, NCH * 32], FP32, tag="aggrbx", bufs=2, name="rb_b")
                    nc.sync.dma_start(
                        rb_b[:].rearrange("p (s f) -> p s f", f=32),
                        agg_d[k][j * N:j * N + N, 0:32].rearrange("(s p) f -> p s f", p=128))
                    dst = agg_rb if j == 3 else rb_a
                    nc.vector.tensor_tensor(dst[:], rb_a[:], rb_b[:], ALU.add)
                agg = vtp.tile([32, N], FP32, tag="agg", bufs=1)
                for g in range(0, NCH, 4):
                    ng = min(4, NCH - g)
                    pt = ps_t.tile([32, 512], FP32, tag="t")
                    for t in range(ng):
                        nc.tensor.transpose(pt[:, 128 * t:128 * (t + 1)],
                                            agg_rb[:, 32 * (g + t):32 * (g + t + 1)], ident)
                    nc.vector.tensor_copy(agg[:, 128 * g:128 * (g + ng)], pt[:, 0:128 * ng])

                # ---- node MLP ----
                V_nxt = vtp.tile([32, N], FP32, tag="vnxt", bufs=2)
                for c0 in range(0, N, 512):
                    nch = min(512, N - c0)
                    sl = slice(c0, c0 + nch)
                    p1 = ps_a.tile([H, 512], FP32, tag="a", name="p1v")
                    nc.tensor.matmul(p1[:, 0:nch], W(f"W1v{k}"), V_cur[0:fv, sl], start=True, stop=False)
                    nc.tensor.matmul(p1[:, 0:nch], W(f"W1g{k}"), agg[:, sl], start=False, stop=True)
                    hv1 = hbufp.tile([H, 512], FP32, tag="h", name="hv1")
                    mlp_ev(p1[:, 0:nch], b1v_ap, hv1[:, 0:nch], "act")
                    p2 = ps_a.tile([H, 512], FP32, tag="a", name="p2v")
                    nc.tensor.matmul(p2[:, 0:nch], W(f"W2v{k}"), hv1[:, 0:nch], start=True, stop=True)
                    hv2 = hbufp.tile([H, 512], FP32, tag="h", name="hv2")
                    mlp_ev(p2[:, 0:nch], W(f"b2v{k}"), hv2[:, 0:nch], "dve")
                    p3 = ps_a.tile([H, 512], FP32, tag="a", name="p3v")
                    nc.tensor.matmul(p3[:, 0:nch], W(f"W3v{k}"), hv2[:, 0:nch], start=True, stop=True)
                    hv3 = hbufp.tile([H, 512], FP32, tag="h", name="hv3")
                    mlp_ev(p3[:, 0:nch], W(f"b3v{k}"), hv3[:, 0:nch], "act")
                    p4 = ps_b.tile([32, 512], FP32, tag="b", name="p4v")
                    nc.tensor.matmul(p4[:, 0:nch], W(f"W4v{k}"), hv3[:, 0:nch], start=True, stop=False)
                    nc.tensor.matmul(p4[:, 0:nch], W(f"b4v{k}"), ones_row[:, 0:nch], start=False, stop=True)
                    nc.scalar.copy(V_nxt[:, sl], p4[:, 0:nch])
                sumV_col = smallp.tile([32, 1], FP32, tag="sumv")
                nc.vector.tensor_reduce(sumV_col[:], V_nxt[:], AX.X, ALU.add)

                # ---- global MLP ----
                p96 = smallp.tile([96, 1], FP32, tag="p96")
                if fu_in:
                    nc.vector.tensor_copy(p96[0:32], u_cur[:])
                else:
                    nc.vector.memset(p96[0:32], 0.0)
                nc.sync.dma_start(p96[32:64], sumV_col[:])
                nc.sync.dma_start(p96[64:96], sumE_col[:])
                pu = col_mlp(p96[:], [(f"W1u_{k}", f"b1u{k}"), (f"W2u{k}", f"b2u{k}"),
                                      (f"W3u{k}", f"b3u{k}"), (f"W4u{k}", f"b4u{k}")], IU)
                pu_row = smallp.tile([1, 32], FP32, tag="purow")
                nc.vector.tensor_copy(pu_row[:], pu[:])
                psU = ps_t.tile([32, 1], FP32, tag="t")
                nc.tensor.transpose(psU[:], pu_row[:], ident[0:1, 0:1])
                u_nxt = smallp.tile([32, 1], FP32, tag=f"u{k}")
                nc.vector.tensor_copy(u_nxt[:], psU[:])

                V_cur, Ea_cur, u_cur = V_nxt, oute, u_nxt

            # ------------- head
            x96 = smallp.tile([96, 1], FP32, tag="x96")
            nc.vector.tensor_copy(x96[0:32], V_cur[:, N - 1:N])
            nc.sync.dma_start(x96[32:64], ones_col[0:32])
            nc.sync.dma_start(x96[64:96], u_cur[:])
            ph = col_mlp(x96[:], [("Wh1", "bh1"), ("Wh2", "bh2"), ("Wh3", "bh3"),
                                  ("Wh4", "bh4")], OUT_EA)
            raw_row = smallp.tile([1, 256], FP32, tag="rawrow", bufs=1)
            nc.vector.memset(raw_row[:], 0.0)
            soft_row = smallp.tile([1, 256], FP32, tag="softrow", bufs=1)
            nc.vector.memset(soft_row[:], 0.0)
            mterm = smallp.tile([1, 256], FP32, tag="mterm", bufs=1)
            nc.vector.tensor_scalar(mterm[0:1, 0:OUT_EA], mh_f[0:1, 0:OUT_EA], 1e10, 1e10, ALU.mult, ALU.subtract)
            nc.vector.tensor_tensor(raw_row[0:1, 0:OUT_EA], ph[:], mterm[0:1, 0:OUT_EA], ALU.add)
            mx = smallp.tile([1, 1], FP32, tag="mx")
            nc.vector.tensor_reduce(mx[:], raw_row[0:1, 0:OUT_EA], AX.X, ALU.max, negate=True)
            ex = smallp.tile([1, 256], FP32, tag="ex", bufs=1)
            sm = smallp.tile([1, 1], FP32, tag="sm")
            nc.scalar.activation(ex[0:1, 0:OUT_EA], raw_row[0:1, 0:OUT_EA], AF.Exp,
                                 bias=mx[:], scale=1.0, accum_out=sm[:])
            rs = smallp.tile([1, 1], FP32, tag="rs")
            nc.vector.reciprocal(rs[:], sm[:])
            nc.vector.tensor_scalar(soft_row[0:1, 0:OUT_EA], ex[0:1, 0:OUT_EA], rs[:], None, ALU.mult)
            nc.sync.dma_start(outD[0:1], raw_row[:])
            nc.sync.dma_start(outD[1:2], soft_row[:])

    nc.finalize()
    return nc


# ------------------------------------------------------------------ host entry
_NC_CACHE = {}


def _get_nc(cfg):
    key = (cfg.N, cfg.E, cfg.n_cores, tuple(cfg.segs))
    if key not in _NC_CACHE:
        _NC_CACHE[key] = build_nc(cfg)
    return _NC_CACHE[key]


def seg_offs(cfg):
    offs = np.zeros(cfg.E, np.int16)
    off = 0
    for q, c in cfg.segs:
        offs[off:off + c] = cfg.N * q
        off += c
    return offs


def wrap16(row):
    E = row.shape[0]
    w = np.zeros((128, E // 16), np.int16)
    w[:16] = row.reshape(E // 16, 16).T
    for rb in range(1, 8):
        w[16 * rb:16 * (rb + 1)] = w[0:16]
    return w


def make_in_maps(cfg, perms, Es, Er, params, Va, Ea, mask_e, mask_v, mask):
    consts = host_consts(cfg)
    wfs, wbs = {}, {}
    for role in (0, 1):
        wfs[role], wbs[role], _ = host_weights(cfg, params, role)
    offs_w = wrap16(seg_offs(cfg))
    in_maps = []
    for c in range(cfg.n_cores):
        b, role = c // 2, c % 2
        p = perms[b]
        real = p >= 0
        pe = np.where(real, p, 0)
        Es_p = np.where(real[None, :], np.asarray(Es[b], np.float32)[:, pe], 0.0).astype(np.float32)
        Er_p = np.where(real[None, :], np.asarray(Er[b], np.float32)[:, pe], 0.0).astype(np.float32)
        Ea_p = np.where(real, np.asarray(Ea[b], np.int64).reshape(-1)[pe], 0).astype(np.int32)
        me_p = np.where(real, np.asarray(mask_e[b], np.uint8)[pe], 0).astype(np.uint8)
        lo, hi = role * cfg.EH, (role + 1) * cfg.EH
        mh = np.ones((1, 256), np.uint8)
        if role == 1:
            mh[0, :OUT_EA] = np.asarray(mask[b], np.uint8)
        in_maps.append({
            "EsH": np.ascontiguousarray(Es_p[:, lo:hi]),
            "ErH": np.ascontiguousarray(Er_p[:, lo:hi]),
            "Va": np.asarray(Va[b], np.int32).reshape(5, cfg.N),
            "Ea": Ea_p.reshape(1, cfg.E),
            "mask_e": me_p.reshape(1, cfg.E),
            "mask_h": mh,
            "offs_w": offs_w,
            "wf32": wfs[role],
            "wbf16": wbs[role],
            "consts": consts,
        })
    return in_maps


def kernel(Es, Er, params, Va, Ea, mask_e, mask_v, mask):
    segs, E_dev, perms = plan_edges(np.asarray(Er))
    cfg = Cfg(N=np.asarray(Es).shape[1], E=E_dev, segs=segs)
    nc = _get_nc(cfg)
    in_maps = make_in_maps(cfg, perms, Es, Er, params, Va, Ea, mask_e, mask_v, mask)
    res = run_bass_kernel_spmd(nc, in_maps, core_ids=list(range(cfg.n_cores)))
    outs = res.results
    B = np.asarray(Es).shape[0]
    val = np.zeros((B, 1), np.float32)
    pol = np.zeros((B, OUT_EA), np.float32)
    for b in range(B):
        val[b, 0] = outs[2 * b]["out"][0, 0]
        pol[b] = outs[2 * b + 1]["out"][1, :OUT_EA]
    return val, pol
